# revision 1
# baseline (speedup 1.0000x reference)
"""Trainium2 Bass kernel for a Neural-CA step (depthwise sobel perceive ->
1x1-conv MLP (48->64->64->16) -> masked update -> alive masking), 2 steps,
batch-sharded across 8 NeuronCores (1 image of [16,256,256] per core).

Layout (per core): x lives in SBUF as [128, 32, 258] fp32 where partition
p = g*16 + c (g = 32-row group, c = channel) and free = (row-in-group,
1 + col) with circularly-padded columns at 0 and 257.

Sobel is computed separably on the vector engine in bf16 (vertical taps via
free-dim shifts + DMA-staged group-boundary halos; horizontal taps via
free-dim shifts). The MLP runs on the tensor engine as bf16 matmuls with
block-diagonal weights packing 2 row-groups per matmul (M=128); layer-1 is
3 PSUM-accumulated matmuls (x*W1id + sx*W1sx + sy*W1sy). The scalar engine
fuses relu+bias into the PSUM->SBUF copies. Alive masks are computed in a
compact 2-rows-per-partition layout and replicated to the x layout via a
DRAM-bounce broadcast DMA.
"""

import numpy as np

import concourse.bacc as bacc
import concourse.mybir as mybir
import concourse.tile as tile
from concourse.bass_utils import run_bass_kernel_spmd

f32 = mybir.dt.float32
bf16 = mybir.dt.bfloat16
AOT = mybir.AluOpType
AF = mybir.ActivationFunctionType

N_CORES = 8
C = 16  # channels
H = W = 256
G = 8  # row groups
RG = H // G  # rows per group (32)
WP = W + 2  # padded width
THRESH = 0.1
UPDATE_RATE = 0.25


def _load_x(nc, xt, x_dram):
    for g in range(G):
        nc.sync.dma_start(
            xt[g * C : (g + 1) * C, :, 1:257], x_dram[:, g * RG : (g + 1) * RG, :]
        )
    nc.gpsimd.tensor_copy(xt[:, :, 0:1], xt[:, :, 256:257])
    nc.gpsimd.tensor_copy(xt[:, :, 257:258], xt[:, :, 1:2])


def _alive_compact(nc, pool, ac, pre, tag_prefix):
    """3x3 circular max-pool of compact alpha ac [128,2,258] (f32, cols 1:257
    valid, pads maintained here), threshold > 0.1 -> pre [128,2,256] bf16."""
    nc.gpsimd.tensor_copy(ac[:, :, 0:1], ac[:, :, 256:257])
    nc.gpsimd.tensor_copy(ac[:, :, 257:258], ac[:, :, 1:2])
    hm = pool.tile([128, 2, 256], f32, name=f"{tag_prefix}_hm", tag="mp_hm")
    hm2 = pool.tile([128, 2, 256], f32, name=f"{tag_prefix}_hm2", tag="mp_hm2")
    nc.vector.tensor_tensor(hm[:], ac[:, :, 0:256], ac[:, :, 1:257], op=AOT.max)
    nc.vector.tensor_tensor(hm2[:], hm[:], ac[:, :, 2:258], op=AOT.max)
    tmp = pool.tile([128, 256], f32, name=f"{tag_prefix}_tmp", tag="mp_tmp")
    nc.vector.tensor_tensor(tmp[:], hm2[:, 0, :], hm2[:, 1, :], op=AOT.max)
    shu = pool.tile([128, 256], f32, name=f"{tag_prefix}_shu", tag="mp_shu")
    shd = pool.tile([128, 256], f32, name=f"{tag_prefix}_shd", tag="mp_shd")
    # shu[p] = hm2[p-1, 1], shd[p] = hm2[p+1, 0] (circular partitions)
    nc.sync.dma_start(shu[1:128], hm2[0:127, 1, :])
    nc.sync.dma_start(shu[0:1], hm2[127:128, 1, :])
    nc.sync.dma_start(shd[0:127], hm2[1:128, 0, :])
    nc.sync.dma_start(shd[127:128], hm2[0:1, 0, :])
    m0 = pool.tile([128, 256], f32, name=f"{tag_prefix}_m0", tag="mp_m0")
    m1 = pool.tile([128, 256], f32, name=f"{tag_prefix}_m1", tag="mp_m1")
    nc.vector.tensor_tensor(m0[:], tmp[:], shu[:], op=AOT.max)
    nc.vector.tensor_tensor(m1[:], tmp[:], shd[:], op=AOT.max)
    nc.vector.tensor_scalar(pre[:, 0, :], m0[:], THRESH, None, op0=AOT.is_gt)
    nc.vector.tensor_scalar(pre[:, 1, :], m1[:], THRESH, None, op0=AOT.is_gt)


def build(steps=2):
    nc = bacc.Bacc(None, target_bir_lowering=False)
    x_dram = nc.dram_tensor("x", [C, H, W], f32, kind="ExternalInput")
    w1_d = nc.dram_tensor("w1", [64, 48], f32, kind="ExternalInput")
    b1_d = nc.dram_tensor("b1", [64], f32, kind="ExternalInput")
    w2_d = nc.dram_tensor("w2", [64, 64], f32, kind="ExternalInput")
    b2_d = nc.dram_tensor("b2", [64], f32, kind="ExternalInput")
    w3_d = nc.dram_tensor("w3", [16, 64], f32, kind="ExternalInput")
    um_d = nc.dram_tensor("um", [steps, H, W], f32, kind="ExternalInput")
    out_d = nc.dram_tensor("out", [C, H, W], f32, kind="ExternalOutput")

    with tile.TileContext(nc) as tc:
        with (
            tc.tile_pool(name="pool", bufs=1) as pool,
            tc.tile_pool(name="hpool", bufs=3) as hpool,
            tc.tile_pool(name="cpool", bufs=1) as cpool,
            tc.tile_pool(name="spool", bufs=2) as spool,
            tc.tile_pool(name="ps1", bufs=4, space="PSUM") as ps1,
            tc.tile_pool(name="ps2", bufs=2, space="PSUM") as ps2,
            tc.tile_pool(name="ps3", bufs=2, space="PSUM") as ps3,
            tc.tile_pool(name="dram", bufs=1, space="DRAM") as dpool,
        ):
            # ---------------- weights ----------------
            # perceive channel order per group: 3c+0=ident, 3c+1=sx, 3c+2=sy
            # L1 weights replicated at partition bases 0/32/64/96 so each
            # group-pair matmul has lhsT.base == rhs.base (row-tiled PE).
            wstage = pool.tile([128, 128], f32, name="wstage", tag="wstage")
            w1xT = pool.tile([128, 128], bf16, name="w1xT")
            w1sxT = pool.tile([128, 128], bf16, name="w1sxT")
            w1syT = pool.tile([128, 128], bf16, name="w1syT")
            w2T = pool.tile([128, 128], bf16, name="w2T")
            w3T = pool.tile([128, 32], bf16, name="w3T")
            for k, wt in ((0, w1xT), (1, w1sxT), (2, w1syT)):
                src = w1_d[:].rearrange("o (i k) -> k i o", k=3)[k]  # [16, 64]
                nc.vector.memset(wstage[:], 0.0)
                for q in range(4):
                    nc.sync.dma_start(wstage[q * 32 : q * 32 + 16, 0:64], src)
                    nc.sync.dma_start(wstage[q * 32 + 16 : q * 32 + 32, 64:128], src)
                nc.vector.tensor_copy(wt[:], wstage[:])
            nc.vector.memset(wstage[:], 0.0)
            w2src = w2_d[:].rearrange("o i -> i o")  # [64, 64]
            nc.sync.dma_start(wstage[0:64, 0:64], w2src)
            nc.sync.dma_start(wstage[64:128, 64:128], w2src)
            nc.vector.tensor_copy(w2T[:], wstage[:])
            nc.vector.memset(wstage[:, 0:32], 0.0)
            w3src = w3_d[:].rearrange("o i -> i o")  # [64, 16]
            nc.sync.dma_start(wstage[0:64, 0:16], w3src)
            nc.sync.dma_start(wstage[64:128, 16:32], w3src)
            nc.vector.tensor_copy(w3T[:], wstage[:, 0:32])
            b1t = pool.tile([128, 1], f32, name="b1t")
            b2t = pool.tile([128, 1], f32, name="b2t")
            nc.sync.dma_start(b1t[0:64, :], b1_d[:].unsqueeze(1))
            nc.sync.dma_start(b1t[64:128, :], b1_d[:].unsqueeze(1))
            nc.sync.dma_start(b2t[0:64, :], b2_d[:].unsqueeze(1))
            nc.sync.dma_start(b2t[64:128, :], b2_d[:].unsqueeze(1))

            # ---------------- state ----------------
            xt = pool.tile([128, RG, WP], f32, name="xt")
            _load_x(nc, xt, x_dram)

            for s in range(steps):
                pfx = f"s{s}"
                # ---- bf16 cast of x (incl pads)
                xb = pool.tile([128, RG, WP], bf16, name=f"{pfx}_xb", tag="xb")
                nc.scalar.copy(xb[:], xt[:])

                # ---- group-boundary halos (circular): hu[p]=xb[p-16,31,:],
                # hd[p]=xb[p+16,0,:]
                hu = pool.tile([128, WP], bf16, name=f"{pfx}_hu", tag="hu")
                hd = pool.tile([128, WP], bf16, name=f"{pfx}_hd", tag="hd")
                nc.sync.dma_start(hu[16:128], xb[0:112, RG - 1, :])
                nc.sync.dma_start(hu[0:16], xb[112:128, RG - 1, :])
                nc.sync.dma_start(hd[0:112], xb[16:128, 0, :])
                nc.sync.dma_start(hd[112:128], xb[0:16, 0, :])

                # sobel emitted per row-chunk with chunk-sized transient
                # tiles (bufs=2) so DVE overlaps the PE matmuls
                def sobel_chunk(r0, r1):
                    n = r1 - r0
                    Ac = spool.tile([128, n, WP], bf16, name=f"A_{s}_{r0}", tag="Ac")
                    t2c = spool.tile([128, n, WP], bf16, name=f"t2_{s}_{r0}", tag="t2c")
                    # vertical: A = up+down, t2 = down-up
                    i0, i1 = max(r0, 1), min(r1, RG - 1)
                    nc.vector.tensor_add(Ac[:, i0 - r0 : i1 - r0, :], xb[:, i0 - 1 : i1 - 1, :], xb[:, i0 + 1 : i1 + 1, :])
                    nc.vector.tensor_sub(t2c[:, i0 - r0 : i1 - r0, :], xb[:, i0 + 1 : i1 + 1, :], xb[:, i0 - 1 : i1 - 1, :])
                    if r0 == 0:
                        nc.vector.tensor_add(Ac[:, 0:1, :], hu[:].unsqueeze(1), xb[:, 1:2, :])
                        nc.vector.tensor_sub(t2c[:, 0:1, :], xb[:, 1:2, :], hu[:].unsqueeze(1))
                    if r1 == RG:
                        nc.vector.tensor_add(Ac[:, n - 1 : n, :], xb[:, RG - 2 : RG - 1, :], hd[:].unsqueeze(1))
                        nc.vector.tensor_sub(t2c[:, n - 1 : n, :], hd[:].unsqueeze(1), xb[:, RG - 2 : RG - 1, :])
                    t1c = spool.tile([128, n, WP], bf16, name=f"t1_{s}_{r0}", tag="t1c")
                    nc.vector.scalar_tensor_tensor(
                        t1c[:], xb[:, r0:r1, :], 2.0, Ac[:], op0=AOT.mult, op1=AOT.add
                    )
                    # horizontal (shifted-by-1 storage: col j = image col j)
                    sxc = spool.tile([128, n, W], bf16, name=f"sx_{s}_{r0}", tag="sxc", bufs=3)
                    nc.vector.tensor_sub(sxc[:], t1c[:, :, 2:258], t1c[:, :, 0:256])
                    sy0c = spool.tile([128, n, W], bf16, name=f"sy0_{s}_{r0}", tag="sy0c")
                    nc.vector.tensor_add(sy0c[:], t2c[:, :, 0:256], t2c[:, :, 2:258])
                    syc = spool.tile([128, n, W], bf16, name=f"sy_{s}_{r0}", tag="syc", bufs=3)
                    nc.vector.scalar_tensor_tensor(
                        syc[:], t2c[:, :, 1:257], 2.0, sy0c[:], op0=AOT.mult, op1=AOT.add
                    )
                    return sxc, syc

                # ---- pre-alive mask from current x (compact alpha layout)
                ac = cpool.tile([128, 2, WP], f32, name=f"{pfx}_ac", tag="ac", bufs=1)
                al_d = dpool.tile([H, W], f32, name=f"{pfx}_al_d", tag="al_d")
                alpha_view = xt[:].rearrange("(g c) r w -> g c r w", c=C)[:, 3, :, 1:257]
                nc.sync.dma_start(
                    al_d[:].rearrange("(g r) w -> g r w", g=G), alpha_view
                )
                nc.sync.dma_start(
                    ac[:, :, 1:257], al_d[:].rearrange("(p r) w -> p r w", r=2)
                )
                pre = cpool.tile([128, 2, W], bf16, name=f"{pfx}_pre", tag="pre", bufs=1)
                _alive_compact(nc, cpool, ac, pre, f"{pfx}pre")

                # ---- update-rate mask (compact)
                umc = cpool.tile([128, 2, W], f32, name=f"{pfx}_umc", tag="umc", bufs=1)
                nc.sync.dma_start(umc[:], um_d[s].rearrange("(p r) w -> p r w", r=2))
                umq = cpool.tile([128, 2, W], bf16, name=f"{pfx}_umq", tag="umq", bufs=1)
                nc.vector.tensor_scalar(umq[:], umc[:], UPDATE_RATE, None, op0=AOT.is_lt)

                # ---- MLP over 2-row slices, 2 groups per matmul (M=128)
                dyf = pool.tile([128, RG, W], bf16, name=f"{pfx}_dyf", tag="dyf")
                # L1 emitted weight-major across the 4 row-tiled group-pairs
                # (consecutive matmuls hit different PE row groups, letting
                # the reorder window pull LDWEIGHTS ahead); L2/L3 for slice
                # k-1 are deferred to interleave with slice k's L1.
                def emit_l1(r2):
                    r = 2 * r2
                    rl = r % 8
                    p1s = [
                        ps1.tile([128, 2, 256], f32, name=f"p1_{s}_{r2}_{gp}", tag="l1")
                        for gp in range(4)
                    ]
                    for gp in range(4):
                        sl = slice(gp * 32, (gp + 1) * 32)
                        tp = (gp * 32, 0)
                        nc.tensor.matmul(p1s[gp][:], w1xT[sl], xb[sl, r : r + 2, 1:257], start=True, stop=False, tile_position=tp)
                        nc.tensor.matmul(p1s[gp][:], w1sxT[sl], sxc[sl, rl : rl + 2, :], start=False, stop=False, tile_position=tp)
                        nc.tensor.matmul(p1s[gp][:], w1syT[sl], syc[sl, rl : rl + 2, :], start=False, stop=True, tile_position=tp)
                    return p1s

                def emit_l23(r2, p1s):
                    r = 2 * r2
                    p3 = ps3.tile([128, 2, 256], f32, name=f"p3_{s}_{r2}", tag="l3")
                    for gp in range(4):
                        h1 = hpool.tile([128, 2, 256], bf16, name=f"h1_{s}_{r2}_{gp}", tag="h1", bufs=4)
                        nc.scalar.activation(h1[:], p1s[gp][:], AF.Relu, bias=b1t[:])
                        p2 = ps2.tile([128, 2, 256], f32, name=f"p2_{s}_{r2}_{gp}", tag="l2")
                        nc.tensor.matmul(p2[:], w2T[:], h1[:], start=True, stop=True)
                        h2 = hpool.tile([128, 2, 256], bf16, name=f"h2_{s}_{r2}_{gp}", tag="h2", bufs=4)
                        nc.scalar.activation(h2[:], p2[:], AF.Relu, bias=b2t[:])
                        nc.tensor.matmul(
                            p3[gp * 32 : (gp + 1) * 32], w3T[:], h2[:],
                            start=True, stop=True, tile_position=(0, gp * 32),
                        )
                    nc.scalar.copy(dyf[:, r : r + 2, :], p3[:])

                sxc = syc = None
                for r2 in range(RG // 2):
                    if (2 * r2) % 8 == 0:
                        sxc, syc = sobel_chunk(2 * r2, 2 * r2 + 8)
                    emit_l23(r2, emit_l1(r2))

                # ---- alpha after unmasked update (compact):
                # alpha_v = alpha + dy[ch3]*umq
                dyA = cpool.tile([128, 2, W], bf16, name=f"{pfx}_dyA", tag="dyA", bufs=1)
                dyA_d = dpool.tile([H, W], bf16, name=f"{pfx}_dyA_d", tag="dyA_d")
                dyA_view = dyf[:].rearrange("(g c) r w -> g c r w", c=C)[:, 3]
                nc.sync.dma_start(
                    dyA_d[:].rearrange("(g r) w -> g r w", g=G), dyA_view
                )
                nc.sync.dma_start(dyA[:], dyA_d[:].rearrange("(p r) w -> p r w", r=2))
                dau = cpool.tile([128, 2, W], f32, name=f"{pfx}_dau", tag="dau", bufs=1)
                nc.vector.tensor_mul(dau[:], dyA[:], umq[:])
                av = cpool.tile([128, 2, WP], f32, name=f"{pfx}_av", tag="av", bufs=1)
                nc.vector.tensor_add(av[:, :, 1:257], ac[:, :, 1:257], dau[:])
                post = cpool.tile([128, 2, W], bf16, name=f"{pfx}_post", tag="post", bufs=1)
                _alive_compact(nc, cpool, av, post, f"{pfx}post")

                # ---- combined masks: a = pre*post, ua = umq*a
                am = cpool.tile([128, 2, W], bf16, name=f"{pfx}_am", tag="am", bufs=1)
                nc.vector.tensor_mul(am[:], pre[:], post[:])
                uam = cpool.tile([128, 2, W], bf16, name=f"{pfx}_uam", tag="uam", bufs=1)
                nc.vector.tensor_mul(uam[:], umq[:], am[:])

                # ---- x = x*a + dy*ua  (per 8-row chunk of every group);
                # masks replicated across the 16 channel-partitions of each
                # group via DVE stream_shuffle (blockwise partition gather:
                # within each 32-partition block, compact row-pair rp lives
                # at in-block partitions rp / 16+rp for the two groups).
                CH = 8
                for cc in range(RG // CH):
                    rr = cc * CH
                    arep = cpool.tile([128, CH, W], bf16, name=f"ar_{s}_{cc}", tag="arep", bufs=2)
                    uarep = cpool.tile([128, CH, W], bf16, name=f"uar_{s}_{cc}", tag="uarep", bufs=2)
                    for j in range(CH // 2):
                        rp = rr // 2 + j
                        mask = [rp] * 16 + [16 + rp] * 16
                        nc.vector.stream_shuffle(
                            arep[:, 2 * j : 2 * j + 2, :], am[:], mask
                        )
                        nc.vector.stream_shuffle(
                            uarep[:, 2 * j : 2 * j + 2, :], uam[:], mask
                        )
                    sA = cpool.tile([128, CH, W], f32, name=f"sA_{s}_{cc}", tag="sA", bufs=1)
                    sB = cpool.tile([128, CH, W], f32, name=f"sB_{s}_{cc}", tag="sB", bufs=1)
                    nc.gpsimd.tensor_mul(sA[:], xt[:, rr : rr + CH, 1:257], arep[:])
                    nc.vector.tensor_mul(sB[:], dyf[:, rr : rr + CH, :], uarep[:])
                    nc.vector.tensor_add(xt[:, rr : rr + CH, 1:257], sA[:], sB[:])

                # ---- refresh circular col pads
                nc.gpsimd.tensor_copy(xt[:, :, 0:1], xt[:, :, 256:257])
                nc.gpsimd.tensor_copy(xt[:, :, 257:258], xt[:, :, 1:2])

            # ---------------- store ----------------
            for g in range(G):
                nc.sync.dma_start(
                    out_d[:, g * RG : (g + 1) * RG, :], xt[g * C : (g + 1) * C, :, 1:257]
                )

    nc.compile()
    return nc


_NC_CACHE = {}


def kernel(**inputs) -> np.ndarray:
    x = np.ascontiguousarray(np.asarray(inputs["x"], dtype=np.float32))
    w1 = np.ascontiguousarray(np.asarray(inputs["w1"], dtype=np.float32))
    b1 = np.ascontiguousarray(np.asarray(inputs["b1"], dtype=np.float32))
    w2 = np.ascontiguousarray(np.asarray(inputs["w2"], dtype=np.float32))
    b2 = np.ascontiguousarray(np.asarray(inputs["b2"], dtype=np.float32))
    w3 = np.ascontiguousarray(np.asarray(inputs["w3"], dtype=np.float32))
    um = np.ascontiguousarray(np.asarray(inputs["update_masks"], dtype=np.float32))
    steps = int(inputs["steps"])
    B = x.shape[0]
    assert B == N_CORES and x.shape == (B, C, H, W)

    if steps not in _NC_CACHE:
        _NC_CACHE[steps] = build(steps)
    nc = _NC_CACHE[steps]

    in_maps = [
        {
            "x": x[b],
            "w1": w1,
            "b1": b1,
            "w2": w2,
            "b2": b2,
            "w3": w3,
            "um": np.ascontiguousarray(um[:, b, 0]),
        }
        for b in range(B)
    ]
    res = run_bass_kernel_spmd(nc, in_maps, core_ids=list(range(N_CORES)))
    return np.stack([res.results[b]["out"] for b in range(B)]).astype(np.float32)



# revision 8
# speedup vs baseline: 1.3137x; 1.3137x over previous
"""Trainium2 Bass kernel for a Neural-CA step (depthwise sobel perceive ->
1x1-conv MLP (48->64->64->16) -> masked update -> alive masking), 2 steps,
batch-sharded across 8 NeuronCores (1 image of [16,256,256] per core).

Layout (per core): x lives in SBUF as bf16 [128, 32, 258] where partition
p = g*16 + c (g = 32-row group, c = channel) and free = (row-in-group,
1 + col) with circularly-padded columns at 0 and 257.

Per step: sobel separably on DVE (bf16); MLP on PE as bf16 matmuls with
block-diagonal weights packing 2 row-groups per matmul (M=128, FD=512);
relu+bias evacuations of paired 2-bank PSUM tiles split between ACT
(activation) and DVE (tensor_scalar add+max); dy is consumed directly from
PSUM: xt += dy*um_replicated in-loop per slice. Alive masks run in a compact
row-pair layout (partition = row-pair); the updated alpha plane is gathered
straight from xt by a partition-strided SBUF DMA; the combined pre*post mask
is replicated across the 16 channel partitions of each group by 16 strided
SBUF->SBUF DMAs, then xt *= mask per row-chunk. Weights arrive pre-tiled
from the host in bf16.
"""

import numpy as np

import bass_rust
import concourse.bacc as bacc
import concourse.mybir as mybir
import concourse.tile as tile
from concourse.bass_utils import run_bass_kernel_spmd

f32 = mybir.dt.float32
bf16 = mybir.dt.bfloat16
AOT = mybir.AluOpType
AF = mybir.ActivationFunctionType

N_CORES = 8
C = 16  # channels
H = W = 256
G = 8  # row groups
RG = H // G  # rows per group (32)
WP = W + 2  # padded width
THRESH = 0.1
UPDATE_RATE = 0.25
CH = 8  # rows per apply/sobel chunk
NCHUNK = RG // CH  # 4


def _dep(nc, a, b):
    """a executes after b (manual edge for DMA APs tile can't track)."""
    bass_rust.add_dep_helper(
        nc.inst_map[a.ins.name], nc.inst_map[b.ins.name], reason="manual"
    )


def _alive_compact(nc, pool, ac, out, tag_prefix):
    """3x3 circular max-pool of compact alpha ac [128,2,258] bf16 (cols 1:257
    valid; col pads written here), threshold > 0.1 -> out [128,2,256] bf16."""
    nc.gpsimd.tensor_copy(ac[:, :, 0:1], ac[:, :, 256:257])
    nc.gpsimd.tensor_copy(ac[:, :, 257:258], ac[:, :, 1:2])
    hm = pool.tile([128, 2, 256], bf16, name=f"{tag_prefix}_hm", tag="mp_hm")
    hm2 = pool.tile([128, 2, 256], bf16, name=f"{tag_prefix}_hm2", tag="mp_hm2")
    nc.vector.tensor_tensor(hm[:], ac[:, :, 0:256], ac[:, :, 1:257], op=AOT.max)
    nc.vector.tensor_tensor(hm2[:], hm[:], ac[:, :, 2:258], op=AOT.max)
    tmp = pool.tile([128, 256], bf16, name=f"{tag_prefix}_tmp", tag="mp_tmp")
    nc.vector.tensor_tensor(tmp[:], hm2[:, 0, :], hm2[:, 1, :], op=AOT.max)
    shu = pool.tile([128, 256], bf16, name=f"{tag_prefix}_shu", tag="mp_shu")
    shd = pool.tile([128, 256], bf16, name=f"{tag_prefix}_shd", tag="mp_shd")
    # shu[p] = hm2[p-1, 1], shd[p] = hm2[p+1, 0] (circular partitions)
    nc.sync.dma_start(shu[1:128], hm2[0:127, 1, :])
    nc.sync.dma_start(shu[0:1], hm2[127:128, 1, :])
    nc.sync.dma_start(shd[0:127], hm2[1:128, 0, :])
    nc.sync.dma_start(shd[127:128], hm2[0:1, 0, :])
    m0 = pool.tile([128, 256], bf16, name=f"{tag_prefix}_m0", tag="mp_m0")
    m1 = pool.tile([128, 256], bf16, name=f"{tag_prefix}_m1", tag="mp_m1")
    nc.vector.tensor_tensor(m0[:], tmp[:], shu[:], op=AOT.max)
    nc.vector.tensor_tensor(m1[:], tmp[:], shd[:], op=AOT.max)
    nc.vector.tensor_scalar(out[:, 0, :], m0[:], THRESH, None, op0=AOT.is_gt)
    nc.vector.tensor_scalar(out[:, 1, :], m1[:], THRESH, None, op0=AOT.is_gt)


def build(steps=2):
    nc = bacc.Bacc(None, target_bir_lowering=False)
    x_dram = nc.dram_tensor("x", [C, H, W], bf16, kind="ExternalInput")
    w1xT_d = nc.dram_tensor("w1xT", [128, 128], bf16, kind="ExternalInput")
    w1sxT_d = nc.dram_tensor("w1sxT", [128, 128], bf16, kind="ExternalInput")
    w1syT_d = nc.dram_tensor("w1syT", [128, 128], bf16, kind="ExternalInput")
    w2T_d = nc.dram_tensor("w2T", [128, 128], bf16, kind="ExternalInput")
    w3T_d = nc.dram_tensor("w3T", [128, 32], bf16, kind="ExternalInput")
    b1_d = nc.dram_tensor("b1t", [128], f32, kind="ExternalInput")
    b2_d = nc.dram_tensor("b2t", [128], f32, kind="ExternalInput")
    um_d = nc.dram_tensor("um", [steps, H, W], f32, kind="ExternalInput")
    out_d = nc.dram_tensor("out", [C, H, W], bf16, kind="ExternalOutput")

    with tile.TileContext(nc) as tc:
        with (
            tc.tile_pool(name="pool", bufs=1) as pool,
            tc.tile_pool(name="hpool", bufs=3) as hpool,
            tc.tile_pool(name="cpool", bufs=1) as cpool,
            tc.tile_pool(name="spool", bufs=2) as spool,
            tc.tile_pool(name="rpool", bufs=2) as rpool,
            tc.tile_pool(name="ps1", bufs=2, space="PSUM") as ps1,
            tc.tile_pool(name="ps2", bufs=1, space="PSUM") as ps2,
            tc.tile_pool(name="ps3", bufs=2, space="PSUM") as ps3,
            tc.tile_pool(name="dram", bufs=2, space="DRAM") as dpool,
        ):
            # ---------------- weights (host-pretiled) ----------------
            w1xT = pool.tile([128, 128], bf16, name="w1xT")
            w1sxT = pool.tile([128, 128], bf16, name="w1sxT")
            w1syT = pool.tile([128, 128], bf16, name="w1syT")
            w2T = pool.tile([128, 128], bf16, name="w2T")
            w3T = pool.tile([128, 32], bf16, name="w3T")
            b1t = pool.tile([128, 1], f32, name="b1t")
            b2t = pool.tile([128, 1], f32, name="b2t")
            for t, d in (
                (w1xT, w1xT_d), (w1sxT, w1sxT_d), (w1syT, w1syT_d),
                (w2T, w2T_d), (w3T, w3T_d),
            ):
                nc.sync.dma_start(t[:], d[:])
            nc.sync.dma_start(b1t[:], b1_d[:].unsqueeze(1))
            nc.sync.dma_start(b2t[:], b2_d[:].unsqueeze(1))

            # ---------------- state ----------------
            xt = pool.tile([128, RG, WP], bf16, name="xt")
            loads = [
                nc.sync.dma_start(
                    xt[g * C : (g + 1) * C, :, 1:257], x_dram[:, g * RG : (g + 1) * RG, :]
                )
                for g in range(G)
            ]
            pad0 = nc.gpsimd.tensor_copy(xt[:, :, 0:1], xt[:, :, 256:257])
            pad1 = nc.gpsimd.tensor_copy(xt[:, :, 257:258], xt[:, :, 1:2])

            prev_muls = []  # step-(s-1) mask-multiply ops (gate next step reads)
            pre = None  # compact pre-alive mask for the current step

            for s in range(steps):
                pfx = f"s{s}"

                # ---- um: load compact f32, threshold, bounce, replicate ----
                umc = cpool.tile([128, 2, W], f32, name=f"{pfx}_umc", tag="umc", bufs=2)
                nc.sync.dma_start(umc[:], um_d[s].rearrange("(p r) w -> p r w", r=2))
                umq = cpool.tile([128, 2, W], bf16, name=f"{pfx}_umq", tag="umq", bufs=2)
                nc.vector.tensor_scalar(umq[:], umc[:], UPDATE_RATE, None, op0=AOT.is_lt)
                umq_d = dpool.tile([H, W], bf16, name=f"{pfx}_umqd", tag="umqd")
                bounce = nc.sync.dma_start(
                    umq_d[:].rearrange("(p j) w -> p j w", j=2), umq[:]
                )
                umr = rpool.tile([128, RG, W], bf16, name=f"{pfx}_umr", tag="umr")
                umr_v = umr[:].rearrange("(g c) r w -> g c r w", c=C)
                um_src = umq_d[:].rearrange("(g r) w -> g r w", g=G)
                umr_dmas = []
                for c in range(C):
                    d = nc.sync.dma_start(umr_v[:, c], um_src)
                    _dep(nc, d, bounce)
                    umr_dmas.append(d)

                # ---- pre-alive mask (step 0 only; later steps compute pre in
                # the previous step's tail from compact av*am) ----
                if s == 0:
                    al_d = dpool.tile([H, W], bf16, name=f"{pfx}_al_d", tag="al_d")
                    alpha_view = xt[:].rearrange("(g c) r w -> g c r w", c=C)[:, 3, :, 1:257]
                    g_ac_w = nc.sync.dma_start(
                        al_d[:].rearrange("(g r) w -> g r w", g=G), alpha_view
                    )
                    for ld in loads:
                        _dep(nc, g_ac_w, ld)
                    ac = cpool.tile([128, 2, WP], bf16, name=f"{pfx}_ac", tag="ac", bufs=1)
                    g_ac = nc.sync.dma_start(
                        ac[:, :, 1:257], al_d[:].rearrange("(p r) w -> p r w", r=2)
                    )
                    _dep(nc, g_ac, g_ac_w)
                    pre = cpool.tile([128, 2, W], bf16, name=f"{pfx}_pre", tag="pre", bufs=1)
                    _alive_compact(nc, cpool, ac, pre, f"{pfx}pre")

                # ---- halos (circular): hu[p]=xt[p-16,31,:], hd[p]=xt[p+16,0,:]
                hu = pool.tile([128, WP], bf16, name=f"{pfx}_hu", tag="hu")
                hd = pool.tile([128, WP], bf16, name=f"{pfx}_hd", tag="hd")
                halo_dmas = [
                    nc.sync.dma_start(hu[16:128], xt[0:112, RG - 1, :]),
                    nc.sync.dma_start(hu[0:16], xt[112:128, RG - 1, :]),
                    nc.sync.dma_start(hd[0:112], xt[16:128, 0, :]),
                    nc.sync.dma_start(hd[112:128], xt[0:16, 0, :]),
                ]
                for hdm in halo_dmas:
                    for m in prev_muls:
                        _dep(nc, hdm, m)

                def sobel_chunk(r0, r1):
                    n = r1 - r0
                    Ac = spool.tile([128, n, WP], bf16, name=f"A_{s}_{r0}", tag="Ac")
                    t2c = spool.tile([128, n, WP], bf16, name=f"t2_{s}_{r0}", tag="t2c")
                    i0, i1 = max(r0, 1), min(r1, RG - 1)
                    nc.vector.tensor_add(Ac[:, i0 - r0 : i1 - r0, :], xt[:, i0 - 1 : i1 - 1, :], xt[:, i0 + 1 : i1 + 1, :])
                    nc.vector.tensor_sub(t2c[:, i0 - r0 : i1 - r0, :], xt[:, i0 + 1 : i1 + 1, :], xt[:, i0 - 1 : i1 - 1, :])
                    if r0 == 0:
                        nc.vector.tensor_add(Ac[:, 0:1, :], hu[:].unsqueeze(1), xt[:, 1:2, :])
                        nc.vector.tensor_sub(t2c[:, 0:1, :], xt[:, 1:2, :], hu[:].unsqueeze(1))
                    if r1 == RG:
                        nc.vector.tensor_add(Ac[:, n - 1 : n, :], xt[:, RG - 2 : RG - 1, :], hd[:].unsqueeze(1))
                        nc.vector.tensor_sub(t2c[:, n - 1 : n, :], hd[:].unsqueeze(1), xt[:, RG - 2 : RG - 1, :])
                    t1c = spool.tile([128, n, WP], bf16, name=f"t1_{s}_{r0}", tag="t1c")
                    nc.vector.scalar_tensor_tensor(
                        t1c[:], xt[:, r0:r1, :], 2.0, Ac[:], op0=AOT.mult, op1=AOT.add
                    )
                    sxc = spool.tile([128, n, W], bf16, name=f"sx_{s}_{r0}", tag="sxc", bufs=2)
                    nc.vector.tensor_sub(sxc[:], t1c[:, :, 2:258], t1c[:, :, 0:256])
                    sy0c = spool.tile([128, n, W], bf16, name=f"sy0_{s}_{r0}", tag="sy0c")
                    nc.vector.tensor_add(sy0c[:], t2c[:, :, 0:256], t2c[:, :, 2:258])
                    syc = spool.tile([128, n, W], bf16, name=f"sy_{s}_{r0}", tag="syc", bufs=2)
                    nc.vector.scalar_tensor_tensor(
                        syc[:], t2c[:, :, 1:257], 2.0, sy0c[:], op0=AOT.mult, op1=AOT.add
                    )
                    return sxc, syc

                # sobel chunk 0 up front; chunk c+1 at the start of chunk c
                # (so its xt reads precede chunk c's in-place updates on DVE)
                sob = [None] * NCHUNK
                sob[0] = sobel_chunk(0, CH)

                plus_ops = []  # per-slice xt += dy*um ops

                for r2 in range(RG // 2):
                    r = 2 * r2
                    cc = r // CH
                    if r % CH == 0 and cc + 1 < NCHUNK:
                        sob[cc + 1] = sobel_chunk((cc + 1) * CH, (cc + 2) * CH)
                    sxc, syc = sob[cc]
                    rl = r % CH

                    # ---- L1: two paired psum tiles (gp 0,1 | gp 2,3)
                    p1s = []
                    for half in range(2):
                        p1 = ps1.tile([128, 4, 256], f32, name=f"p1_{s}_{r2}_{half}", tag="l1")
                        p1s.append(p1)
                        for q in range(2):
                            gp = half * 2 + q
                            sl = slice(gp * 32, (gp + 1) * 32)
                            tp = (gp * 32, 0)
                            o = p1[:, 2 * q : 2 * q + 2, :]
                            nc.tensor.matmul(o, w1xT[sl], xt[sl, r : r + 2, 1:257], start=True, stop=False, tile_position=tp)
                            nc.tensor.matmul(o, w1sxT[sl], sxc[sl, rl : rl + 2, :], start=False, stop=False, tile_position=tp)
                            nc.tensor.matmul(o, w1syT[sl], syc[sl, rl : rl + 2, :], start=False, stop=True, tile_position=tp)

                    # ---- h1 evac (ACT), L2, h2 evac (ACT/DVE), L3
                    p3 = ps3.tile([128, 2, 256], f32, name=f"p3_{s}_{r2}", tag="l3")
                    h1s = []
                    for half in range(2):
                        h1 = hpool.tile([128, 4, 256], bf16, name=f"h1_{s}_{r2}_{half}", tag="h1", bufs=3)
                        nc.scalar.activation(h1[:], p1s[half][:], AF.Relu, bias=b1t[:])
                        h1s.append(h1)
                    for half in range(2):
                        p2 = ps2.tile([128, 4, 256], f32, name=f"p2_{s}_{r2}_{half}", tag="l2")
                        for q in range(2):
                            gp = half * 2 + q
                            nc.tensor.matmul(
                                p2[:, 2 * q : 2 * q + 2, :], w2T[:], h1s[half][:, 2 * q : 2 * q + 2, :],
                                start=True, stop=True,
                            )
                        h2 = hpool.tile([128, 4, 256], bf16, name=f"h2_{s}_{r2}_{half}", tag="h2", bufs=3)
                        if half == 0:
                            nc.scalar.activation(h2[:], p2[:], AF.Relu, bias=b2t[:])
                        else:
                            nc.vector.tensor_scalar(h2[:], p2[:], b2t[:], 0.0, op0=AOT.add, op1=AOT.max)
                        for q in range(2):
                            gp = half * 2 + q
                            nc.tensor.matmul(
                                p3[gp * 32 : (gp + 1) * 32], w3T[:], h2[:, 2 * q : 2 * q + 2, :],
                                start=True, stop=True, tile_position=(0, gp * 32),
                            )

                    # ---- in-place masked update: xt += p3 * umr (no dyf)
                    tmp = spool.tile([128, 2, W], bf16, name=f"tmp_{s}_{r2}", tag="tmp", bufs=2)
                    nc.vector.tensor_tensor(tmp[:], p3[:], umr[:, r : r + 2, :], op=AOT.mult)
                    padd = nc.vector.tensor_add(
                        xt[:, r : r + 2, 1:257], xt[:, r : r + 2, 1:257], tmp[:]
                    )
                    for d in umr_dmas:
                        _dep(nc, padd, d)
                    if s == 0:
                        _dep(nc, padd, g_ac_w)
                    plus_ops.append(padd)

                # ---- tail: av gather (DRAM bounce), post pool, am, replicate,
                # xt *= mask
                av_d = dpool.tile([H, W], bf16, name=f"{pfx}_av_d", tag="av_d")
                alpha_view2 = xt[:].rearrange("(g c) r w -> g c r w", c=C)[:, 3, :, 1:257]
                g_av_w = nc.sync.dma_start(
                    av_d[:].rearrange("(g r) w -> g r w", g=G), alpha_view2
                )
                for p in plus_ops:
                    _dep(nc, g_av_w, p)
                av = cpool.tile([128, 2, WP], bf16, name=f"{pfx}_av", tag="av", bufs=1)
                g_av = nc.sync.dma_start(
                    av[:, :, 1:257], av_d[:].rearrange("(p r) w -> p r w", r=2)
                )
                _dep(nc, g_av, g_av_w)
                post = cpool.tile([128, 2, W], bf16, name=f"{pfx}_post", tag="post", bufs=1)
                _alive_compact(nc, cpool, av, post, f"{pfx}post")
                am = cpool.tile([128, 2, W], bf16, name=f"{pfx}_am", tag="am", bufs=1)
                am_op = nc.vector.tensor_mul(am[:], pre[:], post[:])

                am_d = dpool.tile([H, W], bf16, name=f"{pfx}_am_d", tag="am_d")
                am_w = nc.sync.dma_start(
                    am_d[:].rearrange("(p j) w -> p j w", j=2), am[:]
                )
                _dep(nc, am_w, am_op)
                arep = rpool.tile([128, RG, W], bf16, name=f"{pfx}_ar", tag="arep")
                ar_v = arep[:].rearrange("(g c) r w -> g c r w", c=C)
                am_src = am_d[:].rearrange("(g r) w -> g r w", g=G)
                ar_dmas = []
                for c in range(C):
                    d = nc.sync.dma_start(ar_v[:, c], am_src)
                    _dep(nc, d, am_w)
                    ar_dmas.append(d)

                prev_muls = []
                for cc2 in range(NCHUNK):
                    rr = cc2 * CH
                    mul = nc.vector.tensor_mul(
                        xt[:, rr : rr + CH, 1:257],
                        xt[:, rr : rr + CH, 1:257],
                        arep[:, rr : rr + CH, :],
                    )
                    _dep(nc, mul, g_av_w)
                    for d in ar_dmas:
                        _dep(nc, mul, d)
                    prev_muls.append(mul)

                # ---- next step's pre mask from compact av*am (no gather)
                if s + 1 < steps:
                    acn = cpool.tile([128, 2, WP], bf16, name=f"s{s+1}_ac", tag="acn", bufs=1)
                    nc.vector.tensor_mul(acn[:, :, 1:257], av[:, :, 1:257], am[:])
                    pre = cpool.tile([128, 2, W], bf16, name=f"s{s+1}_pre", tag="pre2", bufs=1)
                    _alive_compact(nc, cpool, acn, pre, f"s{s+1}pre")
                    # refresh circular col pads for next step's sobel
                    rp0 = nc.gpsimd.tensor_copy(xt[:, :, 0:1], xt[:, :, 256:257])
                    rp1 = nc.gpsimd.tensor_copy(xt[:, :, 257:258], xt[:, :, 1:2])

            # ---------------- store ----------------
            for g in range(G):
                st = nc.sync.dma_start(
                    out_d[:, g * RG : (g + 1) * RG, :], xt[g * C : (g + 1) * C, :, 1:257]
                )
                for m in prev_muls:
                    _dep(nc, st, m)

    nc.compile()
    return nc


def _prep_weights(w1, b1, w2, b2, w3):
    """Host-side block-diagonal weight tiling (numpy, bf16 via float32->bf16
    bit truncation-free astype through ml_dtypes if available)."""
    import ml_dtypes

    bf = ml_dtypes.bfloat16

    def w1_part(k):
        src = w1.reshape(64, 16, 3)[:, :, k]  # [64, 16] -> [16, 64] transposed
        t = np.zeros((128, 128), np.float32)
        for q in range(4):
            t[q * 32 : q * 32 + 16, 0:64] = src.T
            t[q * 32 + 16 : q * 32 + 32, 64:128] = src.T
        return t.astype(bf)

    w2T = np.zeros((128, 128), np.float32)
    w2T[0:64, 0:64] = w2.T
    w2T[64:128, 64:128] = w2.T
    w3T = np.zeros((128, 32), np.float32)
    w3T[0:64, 0:16] = w3.T
    w3T[64:128, 16:32] = w3.T
    b1t = np.concatenate([b1, b1]).astype(np.float32)
    b2t = np.concatenate([b2, b2]).astype(np.float32)
    return {
        "w1xT": w1_part(0),
        "w1sxT": w1_part(1),
        "w1syT": w1_part(2),
        "w2T": w2T.astype(bf),
        "w3T": w3T.astype(bf),
        "b1t": b1t,
        "b2t": b2t,
    }


def make_in_maps(inputs):
    import ml_dtypes

    bf = ml_dtypes.bfloat16
    x = np.asarray(inputs["x"], dtype=np.float32)
    um = np.asarray(inputs["update_masks"], dtype=np.float32)
    wts = _prep_weights(
        np.asarray(inputs["w1"], np.float32),
        np.asarray(inputs["b1"], np.float32),
        np.asarray(inputs["w2"], np.float32),
        np.asarray(inputs["b2"], np.float32),
        np.asarray(inputs["w3"], np.float32),
    )
    B = x.shape[0]
    return [
        {
            "x": np.ascontiguousarray(x[b]).astype(bf),
            "um": np.ascontiguousarray(um[:, b, 0]),
            **wts,
        }
        for b in range(B)
    ]


_NC_CACHE = {}


def kernel(**inputs) -> np.ndarray:
    steps = int(inputs["steps"])
    in_maps = make_in_maps(inputs)
    B = len(in_maps)
    assert B == N_CORES

    if steps not in _NC_CACHE:
        _NC_CACHE[steps] = build(steps)
    nc = _NC_CACHE[steps]

    res = run_bass_kernel_spmd(nc, in_maps, core_ids=list(range(N_CORES)))
    return np.stack(
        [np.asarray(res.results[b]["out"], dtype=np.float32) for b in range(B)]
    )
